# revision 1
# baseline (speedup 1.0000x reference)
"""BrokenBiasAttention Trainium2 kernel (8-core SPMD).

Sharding: core c -> batch b=c//2, query-row-half r=c%2 (1024 of 2048 rows).
Each core computes q for its rows, k/v for the whole batch, full 8-head
attention for its rows, and the output projection for its rows. Outputs are
disjoint row blocks -> gather is pure concatenation.

Device algorithm (per core):
  - all matmuls in bf16 (weights/x cast on host)
  - scores^T tiles [krow 128, qrow 512] via row-packed K=32 matmuls
  - softmax without max-subtraction (scores bounded ~|10|), constant shift 20:
      attn_un = exp(s - 20) * expF,   expF = exp(bias) gathered on device
  - bias is 3-level block-Toeplitz: host stages TW[h,rdw,w2,rh,w1] =
      T[h, 4r+rdw, rh, 15+w1-w2]  (pure replication / layout staging);
    device exps it once (small) and DMA-gathers 256-elem contiguous runs to
    build expF[h, rdw, half][128, 256] tiles in SBUF.
  - attn@v + rowsum via column-tiled matmuls accumulating in one PSUM bank
  - normalize: one DVE reciprocal per epilogue + DRAM-bounce broadcast
  - bias-multiply split between DVE and GpSimd.
"""

import math
import sys

import numpy as np

if "/opt/trn_rl_repo" not in sys.path:
    sys.path.insert(0, "/opt/trn_rl_repo")

N = 2048
C = 256
NH = 8
HD = 32
B = 4
QR = 1024  # q rows per core
S_SHIFT = 20.0

_NC = None


def _build_nc(dbg=False):
    import concourse.bass as bass
    import concourse.tile as tile
    from concourse import bacc, mybir
    from concourse.bass import ds, ts

    f32 = mybir.dt.float32
    bf16 = mybir.dt.bfloat16
    EXP = mybir.ActivationFunctionType.Exp

    nc = bacc.Bacc(None, target_bir_lowering=False, debug=False)

    xT = nc.dram_tensor("xT", [C, N], bf16, kind="ExternalInput")
    xTq = nc.dram_tensor("xTq", [C, QR], bf16, kind="ExternalInput")
    Wq_d = nc.dram_tensor("Wq", [C, C], bf16, kind="ExternalInput")
    Wk_d = nc.dram_tensor("Wk", [C, C], bf16, kind="ExternalInput")
    Wv_d = nc.dram_tensor("Wv", [C, C], bf16, kind="ExternalInput")
    Wo_d = nc.dram_tensor("Wo", [C, C], bf16, kind="ExternalInput")
    # TW[h, rdw(11), w2(16), rh(31), w1(16)]
    TW_d = nc.dram_tensor("TW", [NH, 11, 16, 31, 16], f32, kind="ExternalInput")
    out_d = nc.dram_tensor("out", [QR, C], f32, kind="ExternalOutput")

    assert 2 * 11 * 16 * 31 * 16 == 128 * 1364

    with tile.TileContext(nc) as tc:
        with (
            tc.tile_pool(name="consts", bufs=1) as consts,
            tc.tile_pool(name="twp", bufs=2) as twp,
            tc.tile_pool(name="etwp", bufs=2) as etwp,
            tc.tile_pool(name="expfp", bufs=1) as expfp,
            tc.tile_pool(name="xp", bufs=3) as xp,
            tc.tile_pool(name="kqv", bufs=1) as kqv,
            tc.tile_pool(name="ep", bufs=6) as ep,
            tc.tile_pool(name="rp", bufs=2) as rp,
            tc.tile_pool(name="otp", bufs=2) as otp,
            tc.tile_pool(name="stp", bufs=2) as stp,
            tc.tile_pool(name="spsum", bufs=3, space="PSUM") as spsum,
            tc.tile_pool(name="apsum", bufs=2, space="PSUM") as apsum,
            tc.tile_pool(name="dramp", bufs=4, space="DRAM") as dramp,
        ):
            # ---- expF construction: TW -> exp -> dram -> gather ----
            expf_sb = expfp.tile([128, NH * 11 * 384], bf16, tag="expf")
            expf_view = expf_sb.rearrange(
                "p (h r f) -> p h r f", h=NH, r=11, f=384
            )
            etw_d = dramp.tile([4, 128, 1364], bf16, name="etw_d")
            for hp in range(4):
                tw_sb = twp.tile([128, 1364], f32, tag="tw")
                src = TW_d[ds(2 * hp, 2)].rearrange(
                    "h r w2 rh w1 -> (h r w2 rh w1)"
                ).rearrange("(p f) -> p f", p=128)
                nc.scalar.dma_start(out=tw_sb, in_=src)
                etw_sb = etwp.tile([128, 1364], bf16, tag="etw")
                nc.scalar.activation(etw_sb, tw_sb, EXP)
                nc.scalar.dma_start(out=etw_d[hp], in_=etw_sb)
                # gather per h2': dest 16 partitions, free (2*rdw 22, 384)
                # union rh window rows 7-h2' .. 31-h2' (24 rows) covers both halves
                for h2p in range(8):
                    gap = bass.AP(
                        tensor=etw_d.tensor,
                        offset=etw_d.offset + hp * 174592 + (7 - h2p) * 16,
                        ap=[
                            [496, 16],    # w2 (partition)
                            [7936, 22],   # (h in pair, rdw) merged
                            [1, 384],     # (rh-window, w1) contiguous run
                        ],
                    )
                    geng = nc.gpsimd if h2p % 2 == 0 else nc.sync
                    geng.dma_start(
                        out=expf_view[ds(16 * h2p, 16), ds(2 * hp, 2)], in_=gap
                    )

            # ---- constants ----
            w_sb = {}
            for name, d in (("Wq", Wq_d), ("Wk", Wk_d), ("Wv", Wv_d), ("Wo", Wo_d)):
                t = consts.tile([128, 2, C], bf16, tag=f"w_{name}", name=f"w_{name}")
                nc.sync.dma_start(out=t, in_=d[:].rearrange("(ch p) n -> p ch n", p=128))
                w_sb[name] = t
            ones_sb = consts.tile([128, 32], bf16, tag="ones")
            nc.vector.memset(ones_sb, 1.0)
            ebias = consts.tile([128, 1], f32, tag="ebias")
            nc.vector.memset(ebias, -S_SHIFT)

            if dbg:
                dbg_expf = nc.dram_tensor(
                    "dbg_expf", [128, NH * 11 * 384], bf16,
                    kind="ExternalOutput")
                nc.sync.dma_start(out=dbg_expf[:], in_=expf_sb)

            # ---- projections (all bf16) ----
            kT_sb = [kqv.tile([128, N], bf16, tag=f"kT{m}", name=f"kT{m}")
                     for m in range(2)]
            qT_sb = [kqv.tile([128, QR], bf16, tag=f"qT{m}", name=f"qT{m}")
                     for m in range(2)]
            v_sb = kqv.tile([128, 16, C], bf16, tag="v")
            qscale = 1.0 / math.sqrt(HD)

            xTq_r = xTq[:].rearrange("(ch p) n -> p ch n", p=128)
            for j in range(QR // 512):
                xq = xp.tile([128, 2, 512], bf16, tag="x")
                nc.sync.dma_start(out=xq, in_=xTq_r[:, :, ds(512 * j, 512)])
                for m in range(2):
                    ps = spsum.tile([128, 1024], f32, tag="s")
                    for ch in range(2):
                        nc.tensor.matmul(
                            ps[:, :512],
                            lhsT=w_sb["Wq"][:, ch, ts(m, 128)],
                            rhs=xq[:, ch, :],
                            start=(ch == 0),
                            stop=(ch == 1),
                        )
                    nc.vector.tensor_scalar_mul(
                        qT_sb[m][:, ds(512 * j, 512)], ps[:, :512], qscale
                    )

            xT_r = xT[:].rearrange("(ch p) n -> p ch n", p=128)
            for j in range(N // 512):
                xc = xp.tile([128, 2, 512], bf16, tag="x")
                nc.sync.dma_start(out=xc, in_=xT_r[:, :, ds(512 * j, 512)])
                for m in range(2):
                    ps = spsum.tile([128, 1024], f32, tag="s")
                    for ch in range(2):
                        nc.tensor.matmul(
                            ps[:, :512],
                            lhsT=w_sb["Wk"][:, ch, ts(m, 128)],
                            rhs=xc[:, ch, :],
                            start=(ch == 0),
                            stop=(ch == 1),
                        )
                    nc.vector.tensor_copy(kT_sb[m][:, ds(512 * j, 512)], ps[:, :512])
                for t in range(4):
                    kt = 4 * j + t
                    ps = spsum.tile([128, 1024], f32, tag="s")
                    for ch in range(2):
                        nc.tensor.matmul(
                            ps[:, :C],
                            lhsT=xc[:, ch, ts(t, 128)],
                            rhs=w_sb["Wv"][:, ch, :],
                            start=(ch == 0),
                            stop=(ch == 1),
                        )
                    nc.vector.tensor_copy(v_sb[:, kt, :], ps[:, :C])

            # ---- main attention loops ----
            oT_tiles = []
            for qc in range(2):
                oT = otp.tile([128, 2, 512], bf16, tag="oT", name=f"oT{qc}")
                oT_tiles.append(oT)
            for g2 in range(4):
                for qc in range(2):
                    oT = oT_tiles[qc]
                    po_av = 0 if g2 % 2 == 0 else 64
                    po_rs = 64 - po_av
                    half_idx = g2 // 2
                    acc = apsum.tile([128, 512], f32, tag="acc")
                    e_tiles = {}

                    def emit_av(kt):
                        e_t = e_tiles.pop(kt)
                        for k in range(2):
                            h = 2 * g2 + k
                            nc.tensor.matmul(
                                acc[ds(po_av + 32 * k, 32), :],
                                lhsT=v_sb[:, kt, ds(32 * h, 32)],
                                rhs=e_t[:, ts(k, 512)],
                                start=(kt == 0),
                                stop=(kt == 15),
                                tile_position=(0, po_av + 32 * k),
                                skip_group_check=True,
                            )
                            nc.tensor.matmul(
                                acc[ds(po_rs + 32 * k, 32), :],
                                lhsT=ones_sb,
                                rhs=e_t[:, ts(k, 512)],
                                start=(kt == 0),
                                stop=(kt == 15),
                                tile_position=(0, po_rs + 32 * k),
                                skip_group_check=True,
                            )

                    for kt in range(16):
                        s_ps = spsum.tile([128, 1024], f32, tag="s")
                        for k in range(2):
                            h = 2 * g2 + k
                            i = h % 4
                            nc.tensor.matmul(
                                s_ps[:, ts(k, 512)],
                                lhsT=kT_sb[half_idx][ds(32 * i, 32), ts(kt, 128)],
                                rhs=qT_sb[half_idx][ds(32 * i, 32), ts(qc, 512)],
                                start=True,
                                stop=True,
                                tile_position=(32 * i, 0),
                            )
                        e_sb = ep.tile([128, 1024], bf16, tag="e")
                        e_tiles[kt] = e_sb
                        nc.scalar.activation(e_sb, s_ps, EXP, bias=ebias[:, :])
                        rdw0 = 2 * qc - (kt // 2) + 7
                        woff = 128 if kt % 2 == 0 else 0
                        ev = e_sb.rearrange("p (k jj f) -> p k jj f", k=2, jj=2)
                        fv = expf_view[
                            :, ds(2 * g2, 2), ds(rdw0, 2), ds(woff, 256)
                        ]
                        nc.vector.tensor_mul(ev, ev, fv)
                        if kt >= 2:
                            emit_av(kt - 2)
                    emit_av(14)
                    emit_av(15)
                    # epilogue: normalize 2 heads into oT
                    recip = rp.tile([128, 512], f32, tag="recip")
                    rep = rp.tile([128, 512], f32, tag="rep")
                    nc.vector.tensor_copy(
                        recip[ds(po_rs, 64), :], acc[ds(po_rs, 64), :]
                    )
                    nc.vector.reciprocal(
                        recip[ds(po_rs, 64), :], recip[ds(po_rs, 64), :]
                    )
                    nc.sync.dma_start(
                        out=rep[ds(po_av, 64), :], in_=recip[ds(po_rs, 64), :]
                    )
                    nc.vector.tensor_mul(
                        oT[ds(po_av, 64), half_idx, :],
                        acc[ds(po_av, 64), :],
                        rep[ds(po_av, 64), :],
                    )
            # final projections (after both qc loops; off the loop critical path)
            for qc in range(2):
                oT = oT_tiles[qc]
                for s in range(4):
                    fps = spsum.tile([128, 1024], f32, tag="s")
                    for ch in range(2):
                        nc.tensor.matmul(
                            fps[:, :C],
                            lhsT=oT[:, ch, ts(s, 128)],
                            rhs=w_sb["Wo"][:, ch, :],
                            start=(ch == 0),
                            stop=(ch == 1),
                        )
                    stage = stp.tile([128, C], f32, tag="stage")
                    nc.vector.tensor_copy(stage, fps[:, :C])
                    nc.sync.dma_start(
                        out=out_d[ds(512 * qc + 128 * s, 128), :], in_=stage
                    )

    nc.compile()
    return nc


def _host_inputs(x, Wq, Wk, Wv, Wo, bias_table):
    """Build the 8 per-core input maps."""
    import ml_dtypes

    bf = ml_dtypes.bfloat16
    x = np.asarray(x, dtype=np.float32)
    T = np.asarray(bias_table, dtype=np.float32)
    xf = np.ascontiguousarray(x.reshape(B, N, C))
    idx_w = 15 + np.arange(16)[None, :] - np.arange(16)[:, None]  # [w2, w1]
    Ws = {
        "Wq": np.ascontiguousarray(np.asarray(Wq, np.float32).astype(bf)),
        "Wk": np.ascontiguousarray(np.asarray(Wk, np.float32).astype(bf)),
        "Wv": np.ascontiguousarray(np.asarray(Wv, np.float32).astype(bf)),
        "Wo": np.ascontiguousarray(np.asarray(Wo, np.float32).astype(bf)),
    }
    in_maps = []
    for c in range(8):
        b, r = c // 2, c % 2
        d1min = 4 * r
        Twin = T[:, d1min:d1min + 11]                     # [8, 11, 31, 31]
        TW = Twin[:, :, :, idx_w]                         # [8,11,31,16,16] (h,rdw,rh,w2,w1)
        TW = np.ascontiguousarray(TW.transpose(0, 1, 3, 2, 4))  # [h,rdw,w2,rh,w1]
        in_maps.append({
            "xT": np.ascontiguousarray(xf[b].T.astype(bf)),
            "xTq": np.ascontiguousarray(xf[b, QR * r:QR * (r + 1)].T.astype(bf)),
            "TW": TW,
            **Ws,
        })
    return in_maps


def kernel(x, Wq, Wk, Wv, Wo, bias_table, _results_hook=None):
    global _NC
    if _NC is None:
        _NC = _build_nc()
    from concourse.bass_utils import run_bass_kernel_spmd

    in_maps = _host_inputs(x, Wq, Wk, Wv, Wo, bias_table)
    res = run_bass_kernel_spmd(_NC, in_maps, core_ids=list(range(8)))
    if _results_hook is not None:
        _results_hook(res)
    out = np.zeros((B, N, C), dtype=np.float32)
    for c in range(8):
        b, r = c // 2, c % 2
        out[b, QR * r:QR * (r + 1)] = res.results[c]["out"]
    D, H, W = 8, 16, 16
    return out.reshape(B, D, H, W, C)



# revision 10
# speedup vs baseline: 1.0812x; 1.0812x over previous
"""BrokenBiasAttention Trainium2 kernel (8-core SPMD), v2.

Sharding: core c -> batch b=c//2, query-row-half r=c%2 (1024 of 2048 rows).

v2 changes over v1 (253us):
  - Bias tables built fully expanded on HOST and DMA'd contiguously
    (v1 built them on device via ~10k tiny gather descriptors = ~60us of
    pipeline stall at the start).
  - Dual-path exp: head-pairs g2=0..2 use ACT exp + DVE/GpSimd multiply by
    expF (= exp(bias), bf16); head-pair g2=3 uses DVE "fast exp":
        e_i16 = round(A16*s + BPP),  BPP = round(B16 - 20*A16 + A16*bias)
    whose int16 bit pattern IS bf16(exp(s - 20 + bias)) (Schraudolph trick
    at bf16 precision). Per-row softmax normalization cancels the sawtooth
    mean, leaving ~1% element noise (validated: rel err ~0.007).
    Units are interleaved at kt granularity in (A,A,D) groups so ACT and
    DVE exp paths run concurrently.
  - Epilogue reciprocal computed on a [128,8] layout (DRAM-bounce reshape)
    instead of [64,512]: ~0.2us vs 3.3us of DVE per unit.
  - K/Q/V psum->sbuf copies moved to the Scalar engine (idle during setup).
"""

import math
import sys

import numpy as np

if "/opt/trn_rl_repo" not in sys.path:
    sys.path.insert(0, "/opt/trn_rl_repo")

N = 2048
C = 256
NH = 8
HD = 32
B = 4
QR = 1024  # q rows per core
S_SHIFT = 20.0
A16 = 128.0 / math.log(2.0)
B16 = 127.0 * 128.0
INV_A16 = 1.0 / A16

# (g2, qc) units on the DVE fast-exp path (whole softmax rows only)
DVE_G2 = (3,)

_NC = None


def _build_nc(dbg=False):
    import concourse.bass as bass
    import concourse.tile as tile
    from concourse import bacc, mybir
    from concourse.bass import ds, ts

    f32 = mybir.dt.float32
    bf16 = mybir.dt.bfloat16
    i16 = mybir.dt.int16
    EXP = mybir.ActivationFunctionType.Exp

    nc = bacc.Bacc(None, target_bir_lowering=False, debug=False)

    xT = nc.dram_tensor("xT", [C, N], bf16, kind="ExternalInput")
    xTq = nc.dram_tensor("xTq", [C, QR], bf16, kind="ExternalInput")
    Wq_d = nc.dram_tensor("Wq", [C, C], bf16, kind="ExternalInput")
    Wk_d = nc.dram_tensor("Wk", [C, C], bf16, kind="ExternalInput")
    Wv_d = nc.dram_tensor("Wv", [C, C], bf16, kind="ExternalInput")
    Wo_d = nc.dram_tensor("Wo", [C, C], bf16, kind="ExternalInput")
    # host-expanded tables: EXPF bf16 for ACT heads 0..5, BPP i16 for heads 6,7
    EXPF_d = nc.dram_tensor("EXPF", [128, 6 * 11 * 384], bf16, kind="ExternalInput")
    BPP_d = nc.dram_tensor("BPP", [128, 2 * 11 * 384], i16, kind="ExternalInput")
    out_d = nc.dram_tensor("out", [QR, C], f32, kind="ExternalOutput")

    with tile.TileContext(nc) as tc:
        with (
            tc.tile_pool(name="consts", bufs=1) as consts,
            tc.tile_pool(name="xp", bufs=3) as xp,
            tc.tile_pool(name="kqv", bufs=1) as kqv,
            tc.tile_pool(name="ep", bufs=10) as ep,
            tc.tile_pool(name="rp", bufs=2) as rp,
            tc.tile_pool(name="repp", bufs=2) as repp,
            tc.tile_pool(name="otp", bufs=1) as otp,
            tc.tile_pool(name="stp", bufs=2) as stp,
            tc.tile_pool(name="spsum", bufs=2, space="PSUM") as spsum,
            tc.tile_pool(name="apsum", bufs=4, space="PSUM") as apsum,
            tc.tile_pool(name="dramp", bufs=4, space="DRAM") as dramp,
        ):
            # ---- table + weight loads (contiguous, spread over queues) ----
            expf_sb = consts.tile([128, 6 * 11 * 384], bf16, tag="expf")
            bpp_sb = consts.tile([128, 2 * 11 * 384], i16, tag="bpp")
            half_t = 3 * 11 * 384
            nc.scalar.dma_start(out=expf_sb[:, ds(0, half_t)],
                                in_=EXPF_d[:, ds(0, half_t)])
            nc.sync.dma_start(out=expf_sb[:, ds(half_t, half_t)],
                              in_=EXPF_d[:, ds(half_t, half_t)])
            nc.gpsimd.dma_start(out=bpp_sb, in_=BPP_d[:])
            expf_view = expf_sb.rearrange("p (h r f) -> p h r f", h=6, r=11, f=384)
            bpp_view = bpp_sb.rearrange("p (h r f) -> p h r f", h=2, r=11, f=384)

            w_sb = {}
            for name, d in (("Wq", Wq_d), ("Wk", Wk_d), ("Wv", Wv_d), ("Wo", Wo_d)):
                t = consts.tile([128, 2, C], bf16, tag=f"w_{name}", name=f"w_{name}")
                nc.sync.dma_start(out=t, in_=d[:].rearrange("(ch p) n -> p ch n", p=128))
                w_sb[name] = t
            ones_sb = consts.tile([128, 32], bf16, tag="ones")
            nc.vector.memset(ones_sb, 1.0)
            ebias = consts.tile([128, 1], f32, tag="ebias")
            nc.vector.memset(ebias, -S_SHIFT)

            # ---- projections (psum->sbuf copies on Scalar engine) ----
            kT_sb = [kqv.tile([128, N], bf16, tag=f"kT{m}", name=f"kT{m}")
                     for m in range(2)]
            qT_sb = [kqv.tile([128, QR], bf16, tag=f"qT{m}", name=f"qT{m}")
                     for m in range(2)]
            v_sb = kqv.tile([128, 16, C], bf16, tag="v")
            qscale = A16 / math.sqrt(HD)

            xTq_r = xTq[:].rearrange("(ch p) n -> p ch n", p=128)
            for j in range(QR // 512):
                xq = xp.tile([128, 2, 512], bf16, tag="x")
                nc.sync.dma_start(out=xq, in_=xTq_r[:, :, ds(512 * j, 512)])
                for m in range(2):
                    ps = spsum.tile([128, 1024], f32, tag="s")
                    for ch in range(2):
                        nc.tensor.matmul(
                            ps[:, :512],
                            lhsT=w_sb["Wq"][:, ch, ts(m, 128)],
                            rhs=xq[:, ch, :],
                            start=(ch == 0),
                            stop=(ch == 1),
                        )
                    nc.scalar.mul(qT_sb[m][:, ds(512 * j, 512)], ps[:, :512], qscale)

            xT_r = xT[:].rearrange("(ch p) n -> p ch n", p=128)
            for j in range(N // 512):
                xc = xp.tile([128, 2, 512], bf16, tag="x")
                nc.sync.dma_start(out=xc, in_=xT_r[:, :, ds(512 * j, 512)])
                for m in range(2):
                    ps = spsum.tile([128, 1024], f32, tag="s")
                    for ch in range(2):
                        nc.tensor.matmul(
                            ps[:, :512],
                            lhsT=w_sb["Wk"][:, ch, ts(m, 128)],
                            rhs=xc[:, ch, :],
                            start=(ch == 0),
                            stop=(ch == 1),
                        )
                    nc.scalar.copy(kT_sb[m][:, ds(512 * j, 512)], ps[:, :512])
                for t in range(4):
                    kt = 4 * j + t
                    ps = spsum.tile([128, 1024], f32, tag="s")
                    for ch in range(2):
                        nc.tensor.matmul(
                            ps[:, :C],
                            lhsT=xc[:, ch, ts(t, 128)],
                            rhs=w_sb["Wv"][:, ch, :],
                            start=(ch == 0),
                            stop=(ch == 1),
                        )
                    nc.scalar.copy(v_sb[:, kt, :], ps[:, :C])

            # ---- main attention: interleaved unit groups ----
            oT_tiles = [otp.tile([128, 2, 512], bf16, tag="oT", name=f"oT{qc}")
                        for qc in range(2)]

            # groups of (g2, qc, path); path 'A' = ACT exp, 'D' = DVE fast-exp
            groups = [
                [(0, 0, "A"), (1, 0, "A"), (3, 0, "D")],
                [(2, 0, "A"), (2, 1, "A")],
                [(0, 1, "A"), (1, 1, "A"), (3, 1, "D")],
            ]

            def make_unit(g2, qc, path, mult_eng):
                u = dict(g2=g2, qc=qc, path=path, mult_eng=mult_eng,
                         half=g2 // 2, po_av=0 if g2 % 2 == 0 else 64)
                u["po_rs"] = 64 - u["po_av"]
                u["acc"] = apsum.tile([128, 512], f32, tag="acc", name=f"acc{g2}_{qc}")
                u["e_tiles"] = {}
                return u

            def emit_scores(u, kt):
                s_ps = spsum.tile([128, 1024], f32, tag="s")
                g2, qc, half = u["g2"], u["qc"], u["half"]
                for k in range(2):
                    h = 2 * g2 + k
                    i = h % 4
                    nc.tensor.matmul(
                        s_ps[:, ts(k, 512)],
                        lhsT=kT_sb[half][ds(32 * i, 32), ts(kt, 128)],
                        rhs=qT_sb[half][ds(32 * i, 32), ts(qc, 512)],
                        start=True,
                        stop=True,
                        tile_position=(32 * i, 0),
                    )
                return s_ps

            def emit_elem(u, kt, s_ps):
                g2, qc = u["g2"], u["qc"]
                rdw0 = 2 * qc - (kt // 2) + 7
                woff = 128 if kt % 2 == 0 else 0
                if u["path"] == "A":
                    e_sb = ep.tile([128, 1024], bf16, tag="e")
                    nc.scalar.activation(e_sb, s_ps, EXP, bias=ebias[:, :],
                                         scale=INV_A16)
                    ev = e_sb.rearrange("p (k jj f) -> p k jj f", k=2, jj=2)
                    fv = expf_view[:, ds(2 * g2, 2), ds(rdw0, 2), ds(woff, 256)]
                    u["mult_eng"].tensor_mul(ev, ev, fv)
                else:
                    e_sb = ep.tile([128, 1024], i16, tag="e")
                    ev = e_sb.rearrange("p (k jj f) -> p k jj f", k=2, jj=2)
                    sv = s_ps.rearrange("p (k jj f) -> p k jj f", k=2, jj=2)
                    bv = bpp_view[:, ds(0, 2), ds(rdw0, 2), ds(woff, 256)]
                    nc.vector.tensor_add(ev, sv, bv)
                u["e_tiles"][kt] = e_sb

            def emit_av(u, kt):
                e_t = u["e_tiles"].pop(kt)
                acc, g2 = u["acc"], u["g2"]
                po_av, po_rs = u["po_av"], u["po_rs"]
                for k in range(2):
                    h = 2 * g2 + k
                    rhs = e_t[:, ts(k, 512)]
                    if u["path"] == "D":
                        rhs = rhs.bitcast(bf16)
                    nc.tensor.matmul(
                        acc[ds(po_av + 32 * k, 32), :],
                        lhsT=v_sb[:, kt, ds(32 * h, 32)],
                        rhs=rhs,
                        start=(kt == 0),
                        stop=(kt == 15),
                        tile_position=(0, po_av + 32 * k),
                        skip_group_check=True,
                    )
                    nc.tensor.matmul(
                        acc[ds(po_rs + 32 * k, 32), :],
                        lhsT=ones_sb,
                        rhs=rhs,
                        start=(kt == 0),
                        stop=(kt == 15),
                        tile_position=(0, po_rs + 32 * k),
                        skip_group_check=True,
                    )

            def emit_epilogue(u):
                acc, g2, qc = u["acc"], u["g2"], u["qc"]
                po_av, po_rs = u["po_av"], u["po_rs"]
                # rowsums: adjacent rows po_rs+31 (h0), po_rs+32 (h1) -> SBUF
                # (DMA cannot read PSUM; DVE cannot partition-shift, so copy
                # in place then bounce via DRAM to reshape to [128, 8])
                rsc = repp.tile([128, 512], f32, tag="rsc")
                nc.vector.tensor_copy(rsc[ds(po_rs, 64), :],
                                      acc[ds(po_rs, 64), :])
                rsb_d = dramp.tile([2, 512], f32, tag="rsb_d")
                for k in range(2):
                    nc.sync.dma_start(out=rsb_d[ds(k, 1)],
                                      in_=rsc[ds(po_rs + 32 * k, 1), :])
                rsb = rp.tile([128, 8], f32, tag="rsb")
                nc.sync.dma_start(
                    out=rsb, in_=rsb_d[:].rearrange("h (p f) -> (h p) f", f=8))
                nc.vector.reciprocal(rsb, rsb)
                rs2_d = dramp.tile([128, 8], f32, tag="rs2_d")
                nc.sync.dma_start(out=rs2_d, in_=rsb)
                rep = repp.tile([128, 512], f32, tag="rep")
                for k in range(2):
                    src = bass.AP(
                        tensor=rs2_d.tensor,
                        offset=rs2_d.offset + 512 * k,
                        ap=[[0, 32], [1, 512]],
                    )
                    nc.sync.dma_start(out=rep[ds(po_av + 32 * k, 32), :], in_=src)
                nc.vector.tensor_mul(
                    oT_tiles[qc][ds(po_av, 64), u["half"], :],
                    acc[ds(po_av, 64), :],
                    rep[ds(po_av, 64), :],
                )

            for gspec in groups:
                units = []
                ai = 0
                for (g2, qc, path) in gspec:
                    if path == "A":
                        # in 3-unit groups, second A unit's multiply -> GpSimd
                        eng = nc.gpsimd if (len(gspec) == 3 and ai == 1) else nc.vector
                        ai += 1
                    else:
                        eng = nc.vector
                    units.append(make_unit(g2, qc, path, eng))
                for kt in range(16):
                    for u in units:
                        s_ps = emit_scores(u, kt)
                        emit_elem(u, kt, s_ps)
                    if kt >= 2:
                        for u in units:
                            emit_av(u, kt - 2)
                for u in units:
                    emit_av(u, 14)
                    emit_av(u, 15)
                for u in units:
                    emit_epilogue(u)

            # ---- final output projections ----
            for qc in range(2):
                oT = oT_tiles[qc]
                for s in range(4):
                    fps = spsum.tile([128, 1024], f32, tag="s")
                    for ch in range(2):
                        nc.tensor.matmul(
                            fps[:, :C],
                            lhsT=oT[:, ch, ts(s, 128)],
                            rhs=w_sb["Wo"][:, ch, :],
                            start=(ch == 0),
                            stop=(ch == 1),
                        )
                    stage = stp.tile([128, C], f32, tag="stage")
                    nc.scalar.copy(stage, fps[:, :C])
                    nc.sync.dma_start(
                        out=out_d[ds(512 * qc + 128 * s, 128), :], in_=stage
                    )

    nc.compile()
    return nc


def _host_tables(T):
    """Expanded Toeplitz tables for both row-halves rr=0,1.

    expf[p, h, r, f] = exp(T[h, 4*rr+r, 7 - p//16 + f//16, 15 + f%16 - p%16])
    (mapping mirrors the v1 on-device gather APs; heads 0..5 -> bf16 exp
    factors, heads 6,7 -> int16 fast-exp bias offsets.)
    """
    import ml_dtypes

    bf = ml_dtypes.bfloat16
    p = np.arange(128)
    f = np.arange(384)
    rh_idx = 7 - p[:, None] // 16 + f[None, :] // 16     # [128, 384]
    w_idx = 15 + f[None, :] % 16 - (p[:, None] % 16)     # [128, 384]
    out = {}
    for rr in (0, 1):
        Twin = T[:, 4 * rr:4 * rr + 11]                  # [8, 11, 31, 31]
        T2 = Twin[:, :, rh_idx, w_idx]                   # [8, 11, 128, 384]
        T2 = np.ascontiguousarray(T2.transpose(2, 0, 1, 3))  # [128, 8, 11, 384]
        expf = np.exp(T2[:, :6]).astype(bf).reshape(128, -1)
        bpp = np.round(B16 - S_SHIFT * A16 + A16 * T2[:, 6:8]).astype(
            np.int16).reshape(128, -1)
        out[rr] = (np.ascontiguousarray(expf), np.ascontiguousarray(bpp))
    return out


def _host_inputs(x, Wq, Wk, Wv, Wo, bias_table):
    """Build the 8 per-core input maps."""
    import ml_dtypes

    bf = ml_dtypes.bfloat16
    x = np.asarray(x, dtype=np.float32)
    T = np.asarray(bias_table, dtype=np.float32)
    xf = np.ascontiguousarray(x.reshape(B, N, C))
    tabs = _host_tables(T)
    Ws = {
        "Wq": np.ascontiguousarray(np.asarray(Wq, np.float32).astype(bf)),
        "Wk": np.ascontiguousarray(np.asarray(Wk, np.float32).astype(bf)),
        "Wv": np.ascontiguousarray(np.asarray(Wv, np.float32).astype(bf)),
        "Wo": np.ascontiguousarray(np.asarray(Wo, np.float32).astype(bf)),
    }
    in_maps = []
    for c in range(8):
        b, r = c // 2, c % 2
        expf, bpp = tabs[r]
        in_maps.append({
            "xT": np.ascontiguousarray(xf[b].T.astype(bf)),
            "xTq": np.ascontiguousarray(xf[b, QR * r:QR * (r + 1)].T.astype(bf)),
            "EXPF": expf,
            "BPP": bpp,
            **Ws,
        })
    return in_maps


def kernel(x, Wq, Wk, Wv, Wo, bias_table, _results_hook=None):
    global _NC
    if _NC is None:
        _NC = _build_nc()
    from concourse.bass_utils import run_bass_kernel_spmd

    in_maps = _host_inputs(x, Wq, Wk, Wv, Wo, bias_table)
    res = run_bass_kernel_spmd(_NC, in_maps, core_ids=list(range(8)))
    if _results_hook is not None:
        _results_hook(res)
    out = np.zeros((B, N, C), dtype=np.float32)
    for c in range(8):
        b, r = c // 2, c % 2
        out[b, QR * r:QR * (r + 1)] = res.results[c]["out"]
    D, H, W = 8, 16, 16
    return out.reshape(B, D, H, W, C)
